# Initial kernel scaffold
#
"""HGNNPConv Trainium2 kernel (8 NeuronCores, SPMD).

Math (equivalent reformulation of the reference):
  Xe_raw[e] = mean_{i: e_idx[i]=e} X[v_idx[i]]              (v2e, softmax of ones = 1/deg)
  Xe_p      = Xe_raw @ W.T + b                              (GEMM on 4000 edges, not 20000 verts)
  Xv[v]     = sum_i exp(w_i) * Xe_p[e_idx[i]] / sum_i exp(w_i)   over i with v_idx[i]=v
  out       = relu(Xv)
Empty edges get a spurious +b in Xe_p but are never referenced downstream
(an edge appearing in phase 2 has >=1 incidence, hence deg>=1 in phase 1).

Sharding: phase 1 by destination edge (500/core), edge-level GEMM per core,
AllGather of the projected edge table (1MB/core), phase 2 by destination
vertex (2500/core). Per-destination-window weighted one-hot selection
matrices (built on DVE from iota) reduce gathered rows on the PE into PSUM.
"""

import os
from contextlib import ExitStack

import numpy as np
import ml_dtypes

# ---------------------------------------------------------------- config ---
NCORES = 8
NV, NE, NNZ, CH = 20000, 4000, 160000, 512
GATHER_BF16 = os.environ.get("KERNEL_F32", "") == ""  # bf16 tables+matmuls by default
GRP = 8          # gather chunks (of 128 idxs) per dma_gather call
TRACE = os.environ.get("BASS_TRACE", "") != ""

_last_results = None   # BassKernelResults of the most recent run (for test.py)


# ------------------------------------------------------------------- plan ---
class Plan:
    pass


def _binpack(ids, degs, nbins, cap=128):
    """Pack `ids` into `nbins` bins of <=cap items, balancing sum(degs)."""
    import heapq

    order = np.argsort(-degs, kind="stable")
    bins = [[] for _ in range(nbins)]
    loads = [0] * nbins
    heap = [(0, b) for b in range(nbins)]
    heapq.heapify(heap)
    for t in order:
        popped = []
        while True:
            load, b = heapq.heappop(heap)
            if len(bins[b]) < cap:
                break
            popped.append((load, b))
        for p in popped:
            heapq.heappush(heap, p)
        bins[b].append(int(ids[t]))
        loads[b] = load + int(degs[t])
        heapq.heappush(heap, (loads[b], b))
    return bins, loads


def _csr(idx, n):
    order = np.argsort(idx, kind="stable").astype(np.int64)
    deg = np.bincount(idx, minlength=n).astype(np.int64)
    starts = np.zeros(n + 1, np.int64)
    np.cumsum(deg, out=starts[1:])
    return order, deg, starts


def _phase_arrays(bins_per_core, order, starts, W, nw, idx_of_inc, w_of_inc, loc_dtype=np.float32):
    """Per-core flat arrays for one aggregation phase.

    Returns (gidx[int16, 128*W*nw], loc[f32], wsel[f32], members) where slot
    i = (chunk c = i//128, p = i%128), chunk c belongs to window c//W.
    members[w][j] = destination id at window w row j.
    """
    C = nw * W
    L = C * 128
    gidx = np.zeros(L, np.int16)
    loc = np.full(L, -1.0, loc_dtype)
    wsel = np.zeros(L, np.float32)
    members = []
    for w, bin_ids in enumerate(bins_per_core):
        incs = []
        locs = []
        for j, d in enumerate(bin_ids):
            seg = order[starts[d]:starts[d + 1]]
            incs.append(seg)
            locs.append(np.full(len(seg), j, loc_dtype))
        incs = np.concatenate(incs) if incs else np.zeros(0, np.int64)
        locs = np.concatenate(locs) if locs else np.zeros(0, loc_dtype)
        n = len(incs)
        assert n <= W * 128, (n, W)
        o = w * W * 128
        gidx[o:o + n] = idx_of_inc[incs]
        loc[o:o + n] = locs
        wsel[o:o + n] = w_of_inc[incs]
        members.append(bin_ids)
    return gidx, loc, wsel, members


def _wrap_idx(flat):
    """int16 flat[i] -> [128, len/16] with value i at [i%16, i//16], replicated."""
    a = flat.reshape(-1, 16).T  # [16, L/16]
    return np.ascontiguousarray(np.tile(a, (8, 1)))


def _pack(flat, C):
    """flat[c*128+p] -> [128, C]"""
    return np.ascontiguousarray(flat.reshape(C, 128).T)


def make_plan(v_idx, e_idx, e2v_weight, nv=NV, ne=NE, ch=CH, ncores=NCORES):
    P = Plan()
    P.nv, P.ne, P.ch, P.ncores = nv, ne, ch, ncores
    epc, vpc = ne // ncores, nv // ncores
    P.epc, P.vpc = epc, vpc

    order_e, deg_e, starts_e = _csr(e_idx, ne)
    order_v, deg_v, starts_v = _csr(v_idx, nv)
    inv_deg = np.zeros(ne, np.float32)
    nz = deg_e > 0
    inv_deg[nz] = (np.float32(1.0) / deg_e[nz].astype(np.float32))

    nb1 = -(-epc // 128)
    nb2 = -(-vpc // 128)
    bins1, bins2 = [], []
    w1max = w2max = 0
    for k in range(ncores):
        eids = np.arange(k * epc, (k + 1) * epc)
        b, loads = _binpack(eids, deg_e[eids], nb1)
        bins1.append(b)
        w1max = max(w1max, max(loads))
        vids = np.arange(k * vpc, (k + 1) * vpc)
        b, loads = _binpack(vids, deg_v[vids], nb2)
        bins2.append(b)
        w2max = max(w2max, max(loads))
    P.NW1, P.NW2 = nb1, nb2
    P.W1 = -(-w1max // 128)
    P.W2 = -(-w2max // 128)
    P.C1 = P.NW1 * P.W1
    P.C2 = P.NW2 * P.W2

    # phase-1 arrays + edge position map
    pos = np.zeros(ne, np.int64)
    P.p1 = []
    for k in range(ncores):
        gidx, loc, wsel, members = _phase_arrays(
            bins1[k], order_e, starts_e, P.W1, P.NW1, v_idx.astype(np.int64),
            inv_deg[e_idx.astype(np.int64)])
        P.p1.append((gidx, loc, wsel))
        for w, bin_ids in enumerate(members):
            for j, e in enumerate(bin_ids):
                pos[e] = k * P.NW1 * 128 + w * 128 + j
    assert pos.max() < 32768

    # phase-2 arrays + output row map
    P.p2 = []
    P.vmap = []
    for k in range(ncores):
        gidx, loc, wsel, members = _phase_arrays(
            bins2[k], order_v, starts_v, P.W2, P.NW2, pos[e_idx.astype(np.int64)],
            e2v_weight.astype(np.float32))
        P.p2.append((gidx, loc, wsel))
        vm = np.full(P.NW2 * 128, -1, np.int64)
        for w, bin_ids in enumerate(members):
            vm[w * 128:w * 128 + len(bin_ids)] = bin_ids
        P.vmap.append(vm)
    return P


# ---------------------------------------------------------------- builder ---
def build_nc(P, bf16=GATHER_BF16, spmd=True, reps=1, grp=GRP, gbufs=3,
             nqueues=1, no_den=False):
    import concourse.bacc as bacc
    import concourse.mybir as mybir
    import concourse.tile as tile

    f32 = mybir.dt.float32
    dt_g = mybir.dt.bfloat16 if bf16 else f32
    eq, mul, mx, add = (mybir.AluOpType.is_equal, mybir.AluOpType.mult,
                        mybir.AluOpType.max, mybir.AluOpType.add)
    ch, KT = P.ch, P.ch // 128

    nc = bacc.Bacc("TRN2", target_bir_lowering=False, debug=False,
                   num_devices=P.ncores if spmd else 1,
                   num_swdge_queues=nqueues)

    XT = nc.dram_tensor("xt", [P.nv, ch], dt_g, kind="ExternalInput")
    WT = nc.dram_tensor("wt", [128, KT, ch], dt_g, kind="ExternalInput")
    BREP = nc.dram_tensor("brep", [128, ch], f32, kind="ExternalInput")
    IOTA = nc.dram_tensor("iota", [128, 128], f32, kind="ExternalInput")
    IDENT = nc.dram_tensor("ident", [128, 128], f32, kind="ExternalInput")
    G1IDX = nc.dram_tensor("g1idx", [128, P.C1 * 8], mybir.dt.int16, kind="ExternalInput")
    ELOC1 = nc.dram_tensor("eloc1", [128, P.C1], f32, kind="ExternalInput")
    WSEL1 = nc.dram_tensor("wsel1", [128, P.C1], f32, kind="ExternalInput")
    G2IDX = nc.dram_tensor("g2idx", [128, P.C2 * 8], mybir.dt.int16, kind="ExternalInput")
    VLOC2 = nc.dram_tensor("vloc2", [128, P.C2], f32, kind="ExternalInput")
    W2RAW = nc.dram_tensor("w2raw", [128, P.C2], f32, kind="ExternalInput")

    ner1 = P.NW1 * 128
    CCIN = nc.dram_tensor("ccin", [ner1, ch], dt_g)
    CCOUT = nc.dram_tensor("ccout", [P.ncores * ner1, ch], dt_g, addr_space="Shared")
    OUT = nc.dram_tensor("out", [P.NW2 * 128, ch], f32, kind="ExternalOutput")

    with tile.TileContext(nc) as tc, ExitStack() as ctx:
        const = ctx.enter_context(tc.tile_pool(name="const", bufs=1))
        gpool = ctx.enter_context(tc.tile_pool(name="g", bufs=gbufs))
        selp = ctx.enter_context(tc.tile_pool(name="selp", bufs=6))
        psum = ctx.enter_context(tc.tile_pool(name="ps", bufs=2, space="PSUM"))
        sbp = ctx.enter_context(tc.tile_pool(name="sbp", bufs=2))
        xe_pool = ctx.enter_context(tc.tile_pool(name="xe", bufs=1))

        def cload(dram, shape, dt, tag):
            t = const.tile(shape, dt, tag=tag)
            nc.sync.dma_start(t[:], dram[:])
            return t

        wt_t = cload(WT, [128, KT, ch], dt_g, "wt")
        brep_t = cload(BREP, [128, ch], f32, "brep")
        iota_t = cload(IOTA, [128, 128], f32, "iota")
        ident_t = cload(IDENT, [128, 128], f32, "ident")
        g1idx_t = cload(G1IDX, [128, P.C1 * 8], mybir.dt.int16, "g1idx")
        eloc1_t = cload(ELOC1, [128, P.C1], f32, "eloc1")
        wsel1_t = cload(WSEL1, [128, P.C1], f32, "wsel1")
        g2idx_t = cload(G2IDX, [128, P.C2 * 8], mybir.dt.int16, "g2idx")
        vloc2_t = cload(VLOC2, [128, P.C2], f32, "vloc2")
        w2raw_t = cload(W2RAW, [128, P.C2], f32, "w2raw")

        ones_t = const.tile([128, 1], dt_g, tag="ones")
        nc.vector.memset(ones_t[:], 1.0)
        exp_t = const.tile([128, P.C2], f32, tag="exp")
        nc.scalar.activation(exp_t[:], w2raw_t[:], mybir.ActivationFunctionType.Exp)

        # ---------------- phase 1: v2e mean aggregation --------------------
        def agg_phase(src_ap, gidx_t, loc_t, w_t, C, W, gtag, chunk_cb, win_cb):
            pw = None
            for g0 in range(0, C, grp):
                n = min(grp, C - g0)
                gt = gpool.tile([128, n, ch], dt_g, tag=gtag)
                nc.gpsimd.dma_gather(
                    gt[:], src_ap, gidx_t[:, g0 * 8:(g0 + n) * 8],
                    n * 128, n * 128, ch, queue_num=(g0 // grp) % nqueues)
                for j in range(n):
                    c = g0 + j
                    w, cw = divmod(c, W)
                    sel = selp.tile([128, 128], dt_g, tag="sel")
                    nc.vector.tensor_scalar(
                        sel[:], iota_t[:], loc_t[:, c:c + 1], w_t[:, c:c + 1],
                        op0=eq, op1=mul)
                    if cw == 0:
                        pw = psum.tile([128, ch], f32, tag="win")
                    chunk_cb(pw, sel, gt, j, w, cw, cw == W - 1)
                    if cw == W - 1:
                        win_cb(pw, w)

        state = {}

        def p1_chunk(pw, sel, gt, j, w, cw, last):
            nc.tensor.matmul(pw[:], sel[:], gt[:, j, :],
                             start=(cw == 0), stop=last)

        def p1_win(pw, w):
            nc.vector.tensor_copy(state["xe"][:, w, :], pw[:])

        def p2_chunk(pw, sel, gt, j, w, cw, last):
            if cw == 0 and not no_den:
                state["pd"] = psum.tile([128, 1], f32, tag="den", name="pden")
            nc.tensor.matmul(pw[:], sel[:], gt[:, j, :],
                             start=(cw == 0), stop=last)
            if not no_den:
                nc.tensor.matmul(state["pd"][:], sel[:], ones_t[:],
                                 start=(cw == 0), stop=last)

        def p2_win(pw, w):
            if no_den:  # perf-probe only: skip normalization
                ow = sbp.tile([128, ch], f32, tag="ow", name="ow")
                nc.vector.tensor_scalar(ow[:], pw[:], 1.0, 0.0, op0=mul, op1=mx)
                nc.sync.dma_start(OUT[w * 128:(w + 1) * 128, :], ow[:])
                return
            den = sbp.tile([128, 1], f32, tag="den_s", name="den")
            nc.vector.tensor_scalar(den[:], state["pd"][:], 1e-30, None, op0=mx)
            rec = sbp.tile([128, 1], f32, tag="rec", name="rec")
            nc.vector.reciprocal(rec[:], den[:])
            ow = sbp.tile([128, ch], f32, tag="ow", name="ow")
            nc.vector.tensor_scalar(ow[:], pw[:], rec[:, 0:1], 0.0,
                                    op0=mul, op1=mx)
            nc.sync.dma_start(OUT[w * 128:(w + 1) * 128, :], ow[:])

        for _rep in range(reps):
            xe_t = xe_pool.tile([128, P.NW1, ch], f32, tag="xe", name="xe")
            xeT_t = xe_pool.tile([128, KT, ner1], dt_g, tag="xeT", name="xeT")
            state["xe"] = xe_t

            agg_phase(XT[:], g1idx_t, eloc1_t, wsel1_t, P.C1, P.W1, "g1",
                      p1_chunk, p1_win)

            # transpose Xe_raw -> [c_in, e] for the GEMM
            for w in range(P.NW1):
                for k in range(KT):
                    pt = psum.tile([128, 128], f32, tag="aux", name="pt")
                    nc.tensor.transpose(pt[:], xe_t[:, w, k * 128:(k + 1) * 128],
                                        ident_t[:])
                    nc.vector.tensor_copy(xeT_t[:, k, w * 128:(w + 1) * 128], pt[:])

            # GEMM: Xe_p = Xe_raw @ W.T + b  (per 128-edge tile)
            for w in range(P.NW1):
                pg = psum.tile([128, ch], f32, tag="aux", name="pg")
                for k in range(KT):
                    nc.tensor.matmul(pg[:], xeT_t[:, k, w * 128:(w + 1) * 128],
                                     wt_t[:, k, :], start=(k == 0), stop=(k == KT - 1))
                xep = sbp.tile([128, ch], dt_g, tag="xep", name="xep")
                nc.vector.tensor_tensor(xep[:], pg[:], brep_t[:], op=add)
                nc.sync.dma_start(CCIN[w * 128:(w + 1) * 128, :], xep[:])

            if spmd:
                nc.gpsimd.collective_compute(
                    "AllGather", mybir.AluOpType.bypass,
                    replica_groups=[list(range(P.ncores))],
                    ins=[CCIN[:]], outs=[CCOUT[:]])
            else:  # single-core cost-model build: stand-in for the AllGather
                nc.sync.dma_start(CCOUT[0:ner1, :], CCIN[:])

            # phase 2: e2v softmax aggregation (sel weights = exp(w2))
            agg_phase(CCOUT[:], g2idx_t, vloc2_t, exp_t, P.C2, P.W2, "g2",
                      p2_chunk, p2_win)

    nc.compile()
    return nc


# ------------------------------------------------------------------ runner ---
def make_in_maps(P, X, W, b, bf16=GATHER_BF16):
    npdt = ml_dtypes.bfloat16 if bf16 else np.float32
    KT = P.ch // 128
    xt = np.ascontiguousarray(X.astype(npdt))
    wt = np.ascontiguousarray(
        W.T.reshape(KT, 128, P.ch).transpose(1, 0, 2).astype(npdt))
    brep = np.ascontiguousarray(np.broadcast_to(b.astype(np.float32), (128, P.ch)))
    iota = np.ascontiguousarray(
        np.broadcast_to(np.arange(128, dtype=np.float32), (128, 128)))
    ident = np.eye(128, dtype=np.float32)
    in_maps = []
    for k in range(P.ncores):
        g1, l1, w1 = P.p1[k]
        g2, l2, w2 = P.p2[k]
        in_maps.append({
            "xt": xt, "wt": wt, "brep": brep, "iota": iota, "ident": ident,
            "g1idx": _wrap_idx(g1), "eloc1": _pack(l1, P.C1), "wsel1": _pack(w1, P.C1),
            "g2idx": _wrap_idx(g2), "vloc2": _pack(l2, P.C2), "w2raw": _pack(w2, P.C2),
        })
    return in_maps


def assemble(P, shards):
    out = np.zeros((P.nv, P.ch), np.float32)
    for k in range(P.ncores):
        vm = P.vmap[k]
        m = vm >= 0
        out[vm[m]] = shards[k][m]
    return out


_nc_cache = {}


def kernel(X, W, b, e2v_weight, v_idx, e_idx):
    global _last_results
    from concourse.bass_utils import run_bass_kernel_spmd

    P = make_plan(v_idx, e_idx, e2v_weight)
    key = (P.C1, P.C2, P.W1, P.W2, GATHER_BF16)
    if key not in _nc_cache:
        _nc_cache[key] = build_nc(P)
    nc = _nc_cache[key]
    in_maps = make_in_maps(P, X, W, b)
    res = run_bass_kernel_spmd(nc, in_maps, list(range(P.ncores)), trace=TRACE)
    _last_results = res
    shards = [res.results[k]["out"] for k in range(P.ncores)]
    return assemble(P, shards)



# revision 4
# speedup vs baseline: 8.2747x; 8.2747x over previous
"""HGNNPConv Trainium2 kernel (8 NeuronCores, SPMD).

Math (equivalent reformulation of the reference):
  Xe_raw[e] = mean_{i: e_idx[i]=e} X[v_idx[i]]              (v2e, softmax of ones = 1/deg)
  Xe_p      = Xe_raw @ W.T + b                              (GEMM on 4000 edges, not 20000 verts)
  Xv[v]     = sum_i exp(w_i) * Xe_p[e_idx[i]] / sum_i exp(w_i)   over i with v_idx[i]=v
  out       = relu(Xv)
Empty edges get a spurious +b in Xe_p but are never referenced downstream.

Sharding: phase 1 by destination edge (500/core), edge-level GEMM per core,
AllGather of the projected edge table, phase 2 by destination vertex
(2500/core). Per-destination-window one-hot selection matrices (built on DVE
from iota) reduce gathered rows on the PE into PSUM.

v2 changes vs baseline:
  - phase-1 gather + selection matmuls in fp8 e4m3 (values only; sel is an
    EXACT one-hot; 1/deg applied on the Act engine at window drain), paired
    chunks via DoubleRow perf mode (K=256 per matmul).
  - softmax denominators (phase 2) precomputed host-side from the
    quantized weights; window drain = Act Relu with per-partition scale.
  - larger gather groups (fewer SWDGE fixed overheads), 2 DMA queues.
  - bf16 output rows, upcast host-side.
"""

import os
from contextlib import ExitStack

import numpy as np
import ml_dtypes

# ---------------------------------------------------------------- config ---
NCORES = 8
NV, NE, NNZ, CH = 20000, 4000, 160000, 512
P1_FP8 = os.environ.get("KERNEL_P1DT", "bf16") == "fp8"
P2_FP8 = os.environ.get("KERNEL_P2DT", "bf16") == "fp8"
GRP = int(os.environ.get("KERNEL_GRP", "8"))
GBUFS = int(os.environ.get("KERNEL_GBUFS", "3"))
NQUEUES = int(os.environ.get("KERNEL_NQ", "2"))
TRACE = os.environ.get("BASS_TRACE", "") != ""

_last_results = None   # BassKernelResults of the most recent run (for test.py)


# ------------------------------------------------------------------- plan ---
class Plan:
    pass


def _binpack(ids, degs, nbins, cap=128):
    """Pack `ids` into `nbins` bins of <=cap items, balancing sum(degs)."""
    import heapq

    order = np.argsort(-degs, kind="stable")
    bins = [[] for _ in range(nbins)]
    loads = [0] * nbins
    heap = [(0, b) for b in range(nbins)]
    heapq.heapify(heap)
    for t in order:
        popped = []
        while True:
            load, b = heapq.heappop(heap)
            if len(bins[b]) < cap:
                break
            popped.append((load, b))
        for p in popped:
            heapq.heappush(heap, p)
        bins[b].append(int(ids[t]))
        loads[b] = load + int(degs[t])
        heapq.heappush(heap, (loads[b], b))
    return bins, loads


def _csr(idx, n):
    order = np.argsort(idx, kind="stable").astype(np.int64)
    deg = np.bincount(idx, minlength=n).astype(np.int64)
    starts = np.zeros(n + 1, np.int64)
    np.cumsum(deg, out=starts[1:])
    return order, deg, starts


def _phase_arrays(bins_per_core, order, starts, W, nw, idx_of_inc, w_of_inc):
    """Per-core flat arrays for one aggregation phase.

    Returns (gidx[int16, 128*W*nw], loc[f32], wsel[f32], members) where slot
    i = (chunk c = i//128, p = i%128), chunk c belongs to window c//W.
    members[w][j] = destination id at window w row j.
    """
    C = nw * W
    L = C * 128
    gidx = np.zeros(L, np.int16)
    loc = np.full(L, -1.0, np.float32)
    wsel = np.zeros(L, np.float32)
    members = []
    for w, bin_ids in enumerate(bins_per_core):
        incs = []
        locs = []
        for j, d in enumerate(bin_ids):
            seg = order[starts[d]:starts[d + 1]]
            incs.append(seg)
            locs.append(np.full(len(seg), j, np.float32))
        incs = np.concatenate(incs) if incs else np.zeros(0, np.int64)
        locs = np.concatenate(locs) if locs else np.zeros(0, np.float32)
        n = len(incs)
        assert n <= W * 128, (n, W)
        o = w * W * 128
        gidx[o:o + n] = idx_of_inc[incs]
        loc[o:o + n] = locs
        wsel[o:o + n] = w_of_inc[incs]
        members.append(bin_ids)
    return gidx, loc, wsel, members


def _wrap_idx(flat):
    """int16 flat[i] -> [128, len/16] with value i at [i%16, i//16], replicated."""
    a = flat.reshape(-1, 16).T  # [16, L/16]
    return np.ascontiguousarray(np.tile(a, (8, 1)))


def _pack(flat, C):
    """flat[c*128+p] -> [128, C]"""
    return np.ascontiguousarray(flat.reshape(C, 128).T)


def make_plan(v_idx, e_idx, e2v_weight, nv=NV, ne=NE, ch=CH, ncores=NCORES,
              p1_fp8=P1_FP8, p2_fp8=P2_FP8):
    P = Plan()
    P.nv, P.ne, P.ch, P.ncores = nv, ne, ch, ncores
    P.p1_fp8, P.p2_fp8 = p1_fp8, p2_fp8
    epc, vpc = ne // ncores, nv // ncores
    P.epc, P.vpc = epc, vpc

    order_e, deg_e, starts_e = _csr(e_idx, ne)
    order_v, deg_v, starts_v = _csr(v_idx, nv)
    inv_deg = np.zeros(ne, np.float32)
    nz = deg_e > 0
    inv_deg[nz] = (np.float32(1.0) / deg_e[nz].astype(np.float32))

    nb1 = -(-epc // 128)
    nb2 = -(-vpc // 128)
    bins1, bins2 = [], []
    w1max = w2max = 0
    for k in range(ncores):
        eids = np.arange(k * epc, (k + 1) * epc)
        b, loads = _binpack(eids, deg_e[eids], nb1)
        bins1.append(b)
        w1max = max(w1max, max(loads))
        vids = np.arange(k * vpc, (k + 1) * vpc)
        b, loads = _binpack(vids, deg_v[vids], nb2)
        bins2.append(b)
        w2max = max(w2max, max(loads))
    P.NW1, P.NW2 = nb1, nb2
    P.W1 = -(-w1max // 128)
    P.W2 = -(-w2max // 128)
    if p1_fp8 and P.W1 % 2:       # DoubleRow pairs chunks within a window
        P.W1 += 1
    if p2_fp8 and P.W2 % 2:
        P.W2 += 1
    P.C1 = P.NW1 * P.W1
    P.C2 = P.NW2 * P.W2

    # quantized phase-2 softmax weights (host denominator must match device)
    wexp = np.exp(e2v_weight.astype(np.float32)).astype(ml_dtypes.bfloat16)
    if p2_fp8:
        wexp = wexp.astype(ml_dtypes.float8_e4m3).astype(ml_dtypes.bfloat16)
    wexp32 = wexp.astype(np.float32)
    den = np.zeros(nv, np.float32)
    np.add.at(den, v_idx.astype(np.int64), wexp32)
    dinv = np.zeros(nv, np.float32)
    m = den > 0
    dinv[m] = 1.0 / den[m]

    # phase-1 arrays + edge position map + per-window inv_deg
    pos = np.zeros(ne, np.int64)
    P.p1 = []
    P.invd = []
    for k in range(ncores):
        gidx, loc, _, members = _phase_arrays(
            bins1[k], order_e, starts_e, P.W1, P.NW1, v_idx.astype(np.int64),
            np.ones(len(v_idx), np.float32))
        P.p1.append((gidx, loc))
        invd = np.zeros((128, P.NW1), np.float32)
        for w, bin_ids in enumerate(members):
            for j, e in enumerate(bin_ids):
                pos[e] = k * P.NW1 * 128 + w * 128 + j
                invd[j, w] = inv_deg[e]
        P.invd.append(invd)
    assert pos.max() < 32768

    # phase-2 arrays + output row map + per-window 1/den
    P.p2 = []
    P.vmap = []
    P.dinv = []
    for k in range(ncores):
        gidx, loc, wsel, members = _phase_arrays(
            bins2[k], order_v, starts_v, P.W2, P.NW2, pos[e_idx.astype(np.int64)],
            wexp32)
        P.p2.append((gidx, loc, wsel))
        vm = np.full(P.NW2 * 128, -1, np.int64)
        dv = np.zeros((128, P.NW2), np.float32)
        for w, bin_ids in enumerate(members):
            vm[w * 128:w * 128 + len(bin_ids)] = bin_ids
            dv[:len(bin_ids), w] = dinv[bin_ids]
        P.vmap.append(vm)
        P.dinv.append(dv)
    return P


# ---------------------------------------------------------------- builder ---
def build_nc(P, spmd=True, reps=1, grp=GRP, gbufs=GBUFS, nqueues=NQUEUES):
    import concourse.bacc as bacc
    import concourse.mybir as mybir
    import concourse.tile as tile

    f32 = mybir.dt.float32
    bf16 = mybir.dt.bfloat16
    fp8 = mybir.dt.float8e4
    dt1 = fp8 if P.p1_fp8 else bf16
    dt2 = fp8 if P.p2_fp8 else bf16
    eq, mul, add = (mybir.AluOpType.is_equal, mybir.AluOpType.mult,
                    mybir.AluOpType.add)
    DR = mybir.MatmulPerfMode.DoubleRow
    COPY = mybir.ActivationFunctionType.Copy
    RELU = mybir.ActivationFunctionType.Relu
    ch, KT = P.ch, P.ch // 128

    nc = bacc.Bacc("TRN2", target_bir_lowering=False, debug=False,
                   num_devices=P.ncores if spmd else 1,
                   num_swdge_queues=nqueues)

    XT = nc.dram_tensor("xt", [P.nv, ch], dt1, kind="ExternalInput")
    WT = nc.dram_tensor("wt", [128, KT, ch], bf16, kind="ExternalInput")
    BREP = nc.dram_tensor("brep", [128, ch], f32, kind="ExternalInput")
    IOTA = nc.dram_tensor("iota", [128, 128], bf16, kind="ExternalInput")
    IDENT = nc.dram_tensor("ident", [128, 128], f32, kind="ExternalInput")
    G1IDX = nc.dram_tensor("g1idx", [128, P.C1 * 8], mybir.dt.int16, kind="ExternalInput")
    LOC1 = nc.dram_tensor("loc1", [128, P.C1], f32, kind="ExternalInput")
    INVD = nc.dram_tensor("invd", [128, P.NW1], f32, kind="ExternalInput")
    G2IDX = nc.dram_tensor("g2idx", [128, P.C2 * 8], mybir.dt.int16, kind="ExternalInput")
    LOC2 = nc.dram_tensor("loc2", [128, P.C2], f32, kind="ExternalInput")
    WEXP = nc.dram_tensor("wexp", [128, P.C2], f32, kind="ExternalInput")
    DINV = nc.dram_tensor("dinv", [128, P.NW2], f32, kind="ExternalInput")

    ner1 = P.NW1 * 128
    CCIN = nc.dram_tensor("ccin", [ner1, ch], dt2)
    CCOUT = nc.dram_tensor("ccout", [P.ncores * ner1, ch], dt2, addr_space="Shared")
    OUT = nc.dram_tensor("out", [P.NW2 * 128, ch], bf16, kind="ExternalOutput")

    with tile.TileContext(nc) as tc, ExitStack() as ctx:
        const = ctx.enter_context(tc.tile_pool(name="const", bufs=1))
        gpool = ctx.enter_context(tc.tile_pool(name="g", bufs=gbufs))
        selp = ctx.enter_context(tc.tile_pool(name="selp", bufs=6))
        psum = ctx.enter_context(tc.tile_pool(name="ps", bufs=2, space="PSUM"))
        sbp = ctx.enter_context(tc.tile_pool(name="sbp", bufs=2))
        xe_pool = ctx.enter_context(tc.tile_pool(name="xe", bufs=1))

        def cload(dram, shape, dt, tag):
            t = const.tile(shape, dt, tag=tag)
            nc.sync.dma_start(t[:], dram[:])
            return t

        wt_t = cload(WT, [128, KT, ch], bf16, "wt")
        brep_t = cload(BREP, [128, ch], f32, "brep")
        iota_t = cload(IOTA, [128, 128], bf16, "iota")
        ident_t = cload(IDENT, [128, 128], f32, "ident")
        g1idx_t = cload(G1IDX, [128, P.C1 * 8], mybir.dt.int16, "g1idx")
        loc1_t = cload(LOC1, [128, P.C1], f32, "loc1")
        invd_t = cload(INVD, [128, P.NW1], f32, "invd")
        g2idx_t = cload(G2IDX, [128, P.C2 * 8], mybir.dt.int16, "g2idx")
        loc2_t = cload(LOC2, [128, P.C2], f32, "loc2")
        wexp_t = cload(WEXP, [128, P.C2], f32, "wexp")
        dinv_t = cload(DINV, [128, P.NW2], f32, "dinv")

        def agg_phase(src_ap, gidx_t, loc_t, w_t, C, W, dt_g, fp8_pair,
                      gtag, win_cb):
            """Gather rows in groups of `grp` chunks; selection-matmul each
            chunk (or DoubleRow pair) into the window PSUM; drain via win_cb."""
            pw = None
            step = 2 if fp8_pair else 1
            for g0 in range(0, C, grp):
                n = min(grp, C - g0)
                gt = gpool.tile([128, n, ch], dt_g, tag=gtag)
                nc.gpsimd.dma_gather(
                    gt[:], src_ap, gidx_t[:, g0 * 8:(g0 + n) * 8],
                    n * 128, n * 128, ch, queue_num=(g0 // grp) % nqueues)
                for j in range(0, n, step):
                    c = g0 + j
                    w, cw = divmod(c, W)
                    if cw == 0:
                        pw = psum.tile([128, ch], f32, tag="win")
                    if fp8_pair:
                        sel = selp.tile([128, 2, 128], dt_g, tag=gtag + "s")
                        for t in range(2):
                            if w_t is None:
                                nc.vector.tensor_scalar(
                                    sel[:, t, :], iota_t[:],
                                    loc_t[:, c + t:c + t + 1], None, op0=eq)
                            else:
                                nc.vector.tensor_scalar(
                                    sel[:, t, :], iota_t[:],
                                    loc_t[:, c + t:c + t + 1],
                                    w_t[:, c + t:c + t + 1], op0=eq, op1=mul)
                        nc.tensor.matmul(pw[:], sel[:], gt[:, j:j + 2, :],
                                         start=(cw == 0), stop=(cw == W - 2),
                                         perf_mode=DR)
                    else:
                        sel = selp.tile([128, 128], dt_g, tag=gtag + "s")
                        if w_t is None:
                            nc.vector.tensor_scalar(
                                sel[:], iota_t[:], loc_t[:, c:c + 1], None,
                                op0=eq)
                        else:
                            nc.vector.tensor_scalar(
                                sel[:], iota_t[:], loc_t[:, c:c + 1],
                                w_t[:, c:c + 1], op0=eq, op1=mul)
                        nc.tensor.matmul(pw[:], sel[:], gt[:, j, :],
                                         start=(cw == 0), stop=(cw == W - 1))
                    if cw + step == W:
                        win_cb(pw, w)

        state = {}

        def p1_win(pw, w):
            # Xe_raw[:, w, :] = pw * inv_deg  (Act engine; DVE stays on sels)
            nc.scalar.activation(state["xe"][:, w, :], pw[:], COPY,
                                 scale=invd_t[:, w:w + 1])

        def p2_win(pw, w):
            ow = sbp.tile([128, ch], bf16, tag="ow", name="ow")
            nc.scalar.activation(ow[:], pw[:], RELU,
                                 scale=dinv_t[:, w:w + 1])
            nc.sync.dma_start(OUT[w * 128:(w + 1) * 128, :], ow[:])

        for _rep in range(reps):
            xe_t = xe_pool.tile([128, P.NW1, ch], f32, tag="xe", name="xe")
            xeT_t = xe_pool.tile([128, KT, ner1], bf16, tag="xeT", name="xeT")
            state["xe"] = xe_t

            agg_phase(XT[:], g1idx_t, loc1_t, None, P.C1, P.W1, dt1, P.p1_fp8,
                      "g1", p1_win)

            # transpose Xe_raw -> [c_in, e] for the GEMM
            for w in range(P.NW1):
                for k in range(KT):
                    pt = psum.tile([128, 128], f32, tag="aux", name="pt")
                    nc.tensor.transpose(pt[:], xe_t[:, w, k * 128:(k + 1) * 128],
                                        ident_t[:])
                    nc.vector.tensor_copy(xeT_t[:, k, w * 128:(w + 1) * 128], pt[:])

            # GEMM: Xe_p = Xe_raw @ W.T + b  (per 128-edge tile)
            for w in range(P.NW1):
                pg = psum.tile([128, ch], f32, tag="aux", name="pg")
                for k in range(KT):
                    nc.tensor.matmul(pg[:], xeT_t[:, k, w * 128:(w + 1) * 128],
                                     wt_t[:, k, :], start=(k == 0), stop=(k == KT - 1))
                xep = sbp.tile([128, ch], dt2, tag="xep", name="xep")
                nc.vector.tensor_tensor(xep[:], pg[:], brep_t[:], op=add)
                nc.sync.dma_start(CCIN[w * 128:(w + 1) * 128, :], xep[:])

            if spmd:
                nc.gpsimd.collective_compute(
                    "AllGather", mybir.AluOpType.bypass,
                    replica_groups=[list(range(P.ncores))],
                    ins=[CCIN[:]], outs=[CCOUT[:]])
            else:  # single-core cost-model build: stand-in for the AllGather
                nc.sync.dma_start(CCOUT[0:ner1, :], CCIN[:])

            # phase 2: e2v softmax aggregation (sel weights = exp(w2))
            agg_phase(CCOUT[:], g2idx_t, loc2_t, wexp_t, P.C2, P.W2, dt2,
                      P.p2_fp8, "g2", p2_win)

    nc.compile()
    return nc


# ------------------------------------------------------------------ runner ---
def make_in_maps(P, X, W, b):
    bf = ml_dtypes.bfloat16
    np1 = ml_dtypes.float8_e4m3 if P.p1_fp8 else bf
    KT = P.ch // 128
    xt = np.ascontiguousarray(X.astype(np1))
    wt = np.ascontiguousarray(
        W.T.reshape(KT, 128, P.ch).transpose(1, 0, 2).astype(bf))
    brep = np.ascontiguousarray(np.broadcast_to(b.astype(np.float32), (128, P.ch)))
    iota = np.ascontiguousarray(
        np.broadcast_to(np.arange(128, dtype=bf), (128, 128)))
    ident = np.eye(128, dtype=np.float32)
    in_maps = []
    for k in range(P.ncores):
        g1, l1 = P.p1[k]
        g2, l2, w2 = P.p2[k]
        in_maps.append({
            "xt": xt, "wt": wt, "brep": brep, "iota": iota, "ident": ident,
            "g1idx": _wrap_idx(g1), "loc1": _pack(l1, P.C1),
            "invd": P.invd[k],
            "g2idx": _wrap_idx(g2), "loc2": _pack(l2, P.C2),
            "wexp": _pack(w2, P.C2), "dinv": P.dinv[k],
        })
    return in_maps


def assemble(P, shards):
    out = np.zeros((P.nv, P.ch), np.float32)
    for k in range(P.ncores):
        vm = P.vmap[k]
        m = vm >= 0
        out[vm[m]] = shards[k][m].astype(np.float32)
    return out


_nc_cache = {}


def kernel(X, W, b, e2v_weight, v_idx, e_idx):
    global _last_results
    from concourse.bass_utils import run_bass_kernel_spmd

    P = make_plan(v_idx, e_idx, e2v_weight)
    key = (P.C1, P.C2, P.W1, P.W2, P.p1_fp8, P.p2_fp8)
    if key not in _nc_cache:
        _nc_cache[key] = build_nc(P)
    nc = _nc_cache[key]
    in_maps = make_in_maps(P, X, W, b)
    res = run_bass_kernel_spmd(nc, in_maps, list(range(P.ncores)), trace=TRACE)
    _last_results = res
    shards = [res.results[k]["out"] for k in range(P.ncores)]
    return assemble(P, shards)
